# revision 10
# baseline (speedup 1.0000x reference)
"""Trainium2 Bass kernel for nn_ContextClassifier (span classifier + softmax loss).

Data-parallel over labels L=2048 across 8 NeuronCores (256 labels -> 512
feats rows per core). Per core:
  1. indirect-DMA gathers span rows of bf16 `hidden`; PE-transposes (bf16)
     to fp8 ctxT/phrT [128, 8, 256]
  2. FFN via fp8 DoubleRow matmuls + fused tanh -> featsT fp8 [128, 4, 256]
  3. vocab loop: WoT fp8 streamed once in 16 groups of 2000 cols; per
     (m, group) supertile [128 rows, 2000] of logits in PSUM produced by
     fp8 DoubleRow matmuls (contraction 256/instr, 0.5 cyc/row)
  4. sum-exp consumers split across three engine pipelines:
       A: ScalarE exp with fused row-accumulate
       D: DVE Schraudolph exp (affine -> int16 -> bitcast bf16) + 4x-mode
          tensor_scalar accumulate
       P: Pool does the affine/int16 pass, DVE accumulates
  5. exact tag-logit dots on DVE (bf16 gathered Wo rows), per-row loss,
     PE-transpose + reduce to a [4,1] partial
Host combines partials: loss = sum(partials) / (2L + 1e-5).
"""
import math
import os
from contextlib import ExitStack

import numpy as np

import concourse.bass as bass
import concourse.tile as tile
from concourse import bacc, mybir, bass_utils
from concourse.masks import make_identity

# Problem shape (hardcoded per spec)
T, B, H2 = 512, 32, 1024   # seq, batch, 2*hidden
D = 512                    # label_dim
V = 32000                  # vocab
L = 2048                   # labels
K = 2                      # tags per label
NCORES = 8
LSH = L // NCORES          # labels per core = 256
ROWS = 2 * LSH             # feats rows per core = 512
NBLK = LSH // 128          # label blocks of 128 = 2
NM = ROWS // 128           # feats row tiles = 4

GW = 1000                  # vocab group width (2 PSUM banks -> ring depth 4)
NG = V // GW               # 32 groups
CHUNKS = [(0, 512), (512, 488)]
assert CHUNKS[-1][0] + CHUNKS[-1][1] == GW
NST = NG * NM              # 128 supertiles

WOSCALE = 16.0             # Wo prescale (fp8 subnormal avoidance)
A_SCH = float(np.float32(128.0 / math.log(2.0) / WOSCALE))
B_SCH = 16249.125
K_CORR = 1.0               # fine-trim on schraudolph sum at combine

f32 = mybir.dt.float32
bf16 = mybir.dt.bfloat16
fp8 = mybir.dt.float8e4
i16 = mybir.dt.int16
i32 = mybir.dt.int32
AF = mybir.ActivationFunctionType
OP = mybir.AluOpType
PM = mybir.MatmulPerfMode


def _schedule():
    """Assign the NST (g, m) supertiles to consumer pipelines A/D/P.

    Greedy by per-engine load; P costs both DVE (pass1) and Pool (accum).
    """
    nA = int(os.environ.get("K_NA", "68"))
    nD = int(os.environ.get("K_ND", "15"))
    nP = NST - nA - nD
    cost = {"A": {"ACT": 1.287}, "D": {"DVE": 1.602},
            "P": {"DVE": 1.326, "Pool": 1.545}}    # us per supertile
    load = {"ACT": 0.0, "DVE": float(os.environ.get("K_DHEAD", "9")),
            "Pool": float(os.environ.get("K_PHEAD", "10"))}
    cnt = {"A": nA, "D": nD, "P": nP}
    out = []
    for _ in range(NST):
        cands = [e for e in "ADP" if cnt[e] > 0]
        e = min(cands, key=lambda e: max(load[eng] + c
                                         for eng, c in cost[e].items()))
        out.append(e)
        cnt[e] -= 1
        for eng, c in cost[e].items():
            load[eng] += c
    return out


def _build(nc):
    hid_d = nc.dram_tensor("hid", [T * B, H2], bf16, kind="ExternalInput").ap()
    wc8_d = nc.dram_tensor("wc8", [128, 8 * D], fp8, kind="ExternalInput").ap()
    wp8_d = nc.dram_tensor("wp8", [128, 8 * D], fp8, kind="ExternalInput").ap()
    bc_d = nc.dram_tensor("bc", [D], f32, kind="ExternalInput").ap()
    bp_d = nc.dram_tensor("bp", [D], f32, kind="ExternalInput").ap()
    wot_d = nc.dram_tensor("wot", [D, V], fp8, kind="ExternalInput").ap()
    woaug_d = nc.dram_tensor("woaug", [V, 514], bf16, kind="ExternalInput").ap()
    gidx_d = nc.dram_tensor("gidx", [4, LSH], i32, kind="ExternalInput").ap()
    tagidx_d = nc.dram_tensor("tagidx", [K, ROWS], i32, kind="ExternalInput").ap()
    partial_d = nc.dram_tensor("partial", [4, 1], f32, kind="ExternalOutput").ap()

    hid_half = hid_d.rearrange("n (t d) -> (n t) d", t=2)  # [2*T*B, 512] bf16

    sched = _schedule()

    with tile.TileContext(nc) as tc:
        with ExitStack() as ctx:
            pp = ctx.enter_context(tc.tile_pool(name="pp", bufs=1))      # persistent
            gp = ctx.enter_context(tc.tile_pool(name="gp", bufs=2))      # gather tiles
            wp = ctx.enter_context(tc.tile_pool(name="wp", bufs=int(os.environ.get("K_WPB", "4"))))
            ep = ctx.enter_context(tc.tile_pool(name="ep", bufs=int(os.environ.get("K_EPB", "3"))))  # consumer scratch
            sp = ctx.enter_context(tc.tile_pool(name="sp", bufs=4))      # small scratch
            ps = ctx.enter_context(tc.tile_pool(name="ps", bufs=4, space="PSUM"))

            # --- index loads (SP) ---
            idx = [[None] * 4 for _ in range(NBLK)]
            for b in range(NBLK):
                for j in range(4):
                    it = sp.tile([128, 1], i32, tag="gidx", bufs=8, name=f"gidx_t{b}_{j}")
                    nc.sync.dma_start(
                        it[:], gidx_d[j:j + 1, b * 128:(b + 1) * 128].rearrange("a n -> n a"))
                    idx[b][j] = it

            # --- span gathers (Pool SWDGE), bf16 ---
            ctx_ts, phr_ts = [], []
            for b in range(NBLK):
                ctx_lo = gp.tile([128, 512], bf16, tag="ctxlo", name=f"ctx_lo{b}")
                ctx_hi = gp.tile([128, 512], bf16, tag="ctxhi", name=f"ctx_hi{b}")
                nc.gpsimd.indirect_dma_start(
                    out=ctx_lo[:], out_offset=None, in_=hid_half,
                    in_offset=bass.IndirectOffsetOnAxis(ap=idx[b][0][:, :1], axis=0))
                nc.gpsimd.indirect_dma_start(
                    out=ctx_hi[:], out_offset=None, in_=hid_half,
                    in_offset=bass.IndirectOffsetOnAxis(ap=idx[b][1][:, :1], axis=0))
                ctx_ts.append((ctx_lo, ctx_hi))
                beg_t = gp.tile([128, H2], bf16, tag="beg", name=f"beg_t{b}")
                em1_t = gp.tile([128, H2], bf16, tag="em1", name=f"em1_t{b}")
                nc.gpsimd.indirect_dma_start(
                    out=beg_t[:], out_offset=None, in_=hid_d,
                    in_offset=bass.IndirectOffsetOnAxis(ap=idx[b][2][:, :1], axis=0))
                nc.gpsimd.indirect_dma_start(
                    out=em1_t[:], out_offset=None, in_=hid_d,
                    in_offset=bass.IndirectOffsetOnAxis(ap=idx[b][3][:, :1], axis=0))
                phr_t = gp.tile([128, H2], bf16, tag="phr", name=f"phr_t{b}")
                # 0.5x folded into wp8 host-side
                nc.vector.tensor_tensor(out=phr_t[:], in0=beg_t[:], in1=em1_t[:], op=OP.add)
                phr_ts.append(phr_t)

            # --- persistent weights ---
            ident = pp.tile([128, 128], bf16, tag="ident")
            make_identity(nc, ident[:])
            ident8 = pp.tile([128, 128], fp8, tag="ident8")
            make_identity(nc, ident8[:])
            wc8 = pp.tile([128, 8, D], fp8, tag="wc8")
            wp8 = pp.tile([128, 8, D], fp8, tag="wp8")
            nc.sync.dma_start(wc8[:].rearrange("p a b -> p (a b)"), wc8_d)
            nc.sync.dma_start(wp8[:].rearrange("p a b -> p (a b)"), wp8_d)
            bc_sb = pp.tile([128, 4], f32, tag="bc")
            bp_sb = pp.tile([128, 4], f32, tag="bp")
            nc.sync.dma_start(bc_sb[:], bc_d.rearrange("(t p) -> p t", p=128))
            nc.sync.dma_start(bp_sb[:], bp_d.rearrange("(t p) -> p t", p=128))

            # --- transposes to ctxT8/phrT8 [128, 8, 256] fp8 ---
            ctxT8 = pp.tile([128, 8, LSH], fp8, tag="ctxT8")
            phrT8 = pp.tile([128, 8, LSH], fp8, tag="phrT8")
            for b in range(NBLK):
                cols = slice(b * 128, (b + 1) * 128)
                for half in range(2):   # 0: lo (h 0..3), 1: hi (h 4..7)
                    src = ctx_ts[b][half]
                    tp = ps.tile([128, 2048], bf16, tag="ps", name=f"tpc{b}_{half}")
                    for hh in range(4):
                        nc.tensor.transpose(tp[:, hh * 128:(hh + 1) * 128],
                                            src[:, hh * 128:(hh + 1) * 128], ident[:])
                    nc.vector.tensor_copy(
                        ctxT8[:, half * 4:(half + 1) * 4, cols],
                        tp[:, 0:512].rearrange("p (a b) -> p a b", a=4))
            # FFN-c (fp8 DoubleRow) -> featsT_c fp8 [128, 4, 256]
            ftc8 = pp.tile([128, 4, LSH], fp8, tag="ftc8")
            ftp8 = pp.tile([128, 4, LSH], fp8, tag="ftp8")
            for mp in range(4):
                mc = slice(mp * 128, (mp + 1) * 128)
                cps = ps.tile([128, 1024], f32, tag="ps", name=f"cps{mp}")
                for j in range(4):
                    nc.tensor.matmul(cps[:, 0:LSH], wc8[:, 2 * j:2 * j + 2, mc],
                                     ctxT8[:, 2 * j:2 * j + 2, :],
                                     start=(j == 0), stop=(j == 3),
                                     perf_mode=PM.DoubleRow)
                nc.scalar.activation(ftc8[:, mp:mp + 1, :], cps[:, 0:LSH], AF.Tanh,
                                     bias=bc_sb[:, mp:mp + 1])
            # phrase transposes + FFN-p
            for b in range(NBLK):
                cols = slice(b * 128, (b + 1) * 128)
                for half in range(2):
                    tp = ps.tile([128, 2048], bf16, tag="ps", name=f"tpp{b}_{half}")
                    for hh in range(4):
                        h = half * 4 + hh
                        nc.tensor.transpose(tp[:, hh * 128:(hh + 1) * 128],
                                            phr_ts[b][:, h * 128:(h + 1) * 128], ident[:])
                    nc.vector.tensor_copy(
                        phrT8[:, half * 4:(half + 1) * 4, cols],
                        tp[:, 0:512].rearrange("p (a b) -> p a b", a=4))
            for mp in range(4):
                mc = slice(mp * 128, (mp + 1) * 128)
                pps = ps.tile([128, 1024], f32, tag="ps", name=f"pps{mp}")
                for j in range(4):
                    nc.tensor.matmul(pps[:, 0:LSH], wp8[:, 2 * j:2 * j + 2, mc],
                                     phrT8[:, 2 * j:2 * j + 2, :],
                                     start=(j == 0), stop=(j == 3),
                                     perf_mode=PM.DoubleRow)
                nc.scalar.activation(ftp8[:, mp:mp + 1, :], pps[:, 0:LSH], AF.Tanh,
                                     bias=bp_sb[:, mp:mp + 1])

            # --- feats_row bf16 [r][128, D] for the tag path ---
            # (fp8 feats are exactly representable in bf16, so the tag path
            # sees the same feats the vocab matmul uses)
            ftbc = pp.tile([128, 4, LSH], bf16, tag="ftbc")
            ftbp = pp.tile([128, 4, LSH], bf16, tag="ftbp")
            nc.vector.tensor_copy(ftbc[:], ftc8[:])
            nc.vector.tensor_copy(ftbp[:], ftp8[:])
            feats_row = [pp.tile([128, D], bf16, tag=f"frow{r}", name=f"frow{r}")
                         for r in range(NM)]
            for r in range(NM):
                srcb = ftbc if r < 2 else ftbp
                rc = slice((r % 2) * 128, (r % 2) * 128 + 128)
                tp = ps.tile([128, 2048], bf16, tag="ps", name=f"tpf{r}")
                for j in range(4):
                    nc.tensor.transpose(tp[:, j * 128:(j + 1) * 128],
                                        srcb[:, j:j + 1, rc], ident[:])
                nc.vector.tensor_copy(feats_row[r][:], tp[:, 0:512])

            # --- tag inputs (used later, gathered during V-loop) ---
            tagix = [[None] * K for _ in range(NM)]
            for r in range(NM):
                for k in range(K):
                    tix = sp.tile([128, 1], i32, tag="tix", bufs=8, name=f"tix{r}_{k}")
                    nc.sync.dma_start(
                        tix[:], tagidx_d[k:k + 1, r * 128:(r + 1) * 128].rearrange("a n -> n a"))
                    tagix[r][k] = tix

            # --- sum-exp accumulators ---
            sums_a = [pp.tile([128, NG], f32, tag=f"sa{m}", name=f"sa{m}") for m in range(NM)]
            sums_d = [pp.tile([128, NG], f32, tag=f"sd{m}", name=f"sd{m}") for m in range(NM)]
            for m in range(NM):
                nc.vector.memset(sums_a[m][:], 0.0)
                nc.vector.memset(sums_d[m][:], 0.0)

            # --- tag gather + dot emission helpers (interleaved into V-loop) ---
            tl = [[None] * K for _ in range(NM)]
            dv = [[None] * K for _ in range(NM)]
            wtags = {}

            def emit_tag_gather(i):
                r, k = divmod(i, K)
                wt = sp.tile([128, 514], bf16, tag="wtag", bufs=4, name=f"wtag{r}_{k}")
                nc.gpsimd.indirect_dma_start(
                    out=wt[:], out_offset=None, in_=woaug_d,
                    in_offset=bass.IndirectOffsetOnAxis(ap=tagix[r][k][:, :1], axis=0))
                wtags[(r, k)] = wt

            def emit_tag_dot(i):
                r, k = divmod(i, K)
                wt = wtags[(r, k)]
                dvv = pp.tile([128, 1], f32, tag=f"dv{r}_{k}", name=f"dv{r}_{k}")
                nc.vector.tensor_copy(dvv[:], wt[:, 512:513])
                tlv = pp.tile([128, 1], f32, tag=f"tl{r}_{k}", name=f"tl{r}_{k}")
                nc.vector.tensor_tensor_reduce(
                    out=wt[:, 0:D], in0=feats_row[r][:], in1=wt[:, 0:D],
                    scale=1.0, scalar=0.0, op0=OP.mult, op1=OP.add,
                    accum_out=tlv[:])
                tl[r][k] = tlv
                dv[r][k] = dvv

            # --- main vocab loop ---
            NTAG = NM * K
            pool_done = 0
            for g in range(NG):
                v0 = g * GW
                wot_sb = wp.tile([128, 4, GW], fp8, tag="wot")
                nc.sync.dma_start(
                    wot_sb[:],
                    wot_d[:, v0:v0 + GW].rearrange("(k p) c -> p k c", p=128))
                for m in range(NM):
                    st = ps.tile([128, 1024], f32, tag="ps", name=f"st{g}_{m}")
                    lhs = ftc8 if m < 2 else ftp8
                    mcc = slice((m % 2) * 128, (m % 2) * 128 + 128)
                    for (o, w) in CHUNKS:
                        for kp in range(2):
                            nc.tensor.matmul(
                                st[:, o:o + w],
                                lhs[:, 2 * kp:2 * kp + 2, mcc],
                                wot_sb[:, 2 * kp:2 * kp + 2, o:o + w],
                                start=(kp == 0), stop=(kp == 1),
                                perf_mode=PM.DoubleRow)
                    eng = sched[g * NM + m]
                    if eng == "A":
                        ex = ep.tile([128, 1024], bf16, tag="ex", name=f"ex{g}_{m}")
                        nc.scalar.activation(ex[:, 0:GW], st[:, 0:GW], AF.Exp,
                                             scale=1.0 / WOSCALE,
                                             accum_out=sums_a[m][:, g:g + 1])
                    elif eng == "D":
                        p1 = ep.tile([128, 1024], i16, tag="p1d", name=f"p1d{g}_{m}")
                        nc.vector.tensor_scalar(
                            out=p1[:, 0:GW], in0=st[:, 0:GW],
                            scalar1=A_SCH, scalar2=B_SCH,
                            op0=OP.mult, op1=OP.add)
                        pb = p1[:, 0:GW].bitcast(bf16)
                        nc.vector.tensor_scalar(
                            out=pb, in0=pb, scalar1=1.0, scalar2=0.0,
                            op0=OP.mult, op1=OP.add,
                            accum_out=sums_d[m][:, g:g + 1])
                    else:
                        # DVE affine/int16 pass, Pool accumulates from SBUF
                        p1 = ep.tile([128, 1024], i16, tag="p1p", name=f"p1p{g}_{m}")
                        nc.vector.tensor_scalar(
                            out=p1[:, 0:GW], in0=st[:, 0:GW],
                            scalar1=A_SCH, scalar2=B_SCH,
                            op0=OP.mult, op1=OP.add)
                        pb = p1[:, 0:GW].bitcast(bf16)
                        nc.gpsimd.tensor_scalar(
                            out=pb, in0=pb, scalar1=1.0, scalar2=0.0,
                            op0=OP.mult, op1=OP.add,
                            accum_out=sums_d[m][:, g:g + 1])
                # interleave tag gathers/dots during the middle of the loop
                if 3 <= g < 3 + NTAG:
                    emit_tag_gather(g - 3)
                if 6 <= g < 6 + NTAG:
                    emit_tag_dot(g - 6)

            # --- per-row loss ---
            loss_sb = pp.tile([128, NM], f32, tag="loss")
            for m in range(NM):
                ra = sp.tile([128, 1], f32, tag="ra", bufs=8, name=f"ra{m}")
                rd = sp.tile([128, 1], f32, tag="rd", bufs=8, name=f"rd{m}")
                nc.vector.reduce_sum(out=ra[:], in_=sums_a[m][:], axis=mybir.AxisListType.X)
                nc.vector.reduce_sum(out=rd[:], in_=sums_d[m][:], axis=mybir.AxisListType.X)
                tot = sp.tile([128, 1], f32, tag="tot", bufs=8, name=f"tot{m}")
                nc.vector.scalar_tensor_tensor(
                    out=tot[:], in0=rd[:], scalar=K_CORR, in1=ra[:],
                    op0=OP.mult, op1=OP.add)
                lse = sp.tile([128, 1], f32, tag="lse", bufs=8, name=f"lse{m}")
                nc.scalar.activation(lse[:], tot[:], AF.Ln)
                r0 = sp.tile([128, 1], f32, tag="r0", bufs=8, name=f"r0_{m}")
                r1 = sp.tile([128, 1], f32, tag="r1", bufs=8, name=f"r1_{m}")
                nc.vector.tensor_scalar(out=r0[:], in0=dv[m][0][:], scalar1=-1.0,
                                        scalar2=1.0, op0=OP.mult, op1=OP.add)
                nc.vector.tensor_scalar(out=r1[:], in0=dv[m][1][:], scalar1=-1.0,
                                        scalar2=1.0, op0=OP.mult, op1=OP.add)
                n0 = sp.tile([128, 1], f32, tag="n0", bufs=8, name=f"n0_{m}")
                n1 = sp.tile([128, 1], f32, tag="n1", bufs=8, name=f"n1_{m}")
                nc.vector.tensor_tensor(out=n0[:], in0=r0[:], in1=tl[m][0][:], op=OP.mult)
                nc.vector.tensor_tensor(out=n1[:], in0=r1[:], in1=tl[m][1][:], op=OP.mult)
                num = sp.tile([128, 1], f32, tag="num", bufs=8, name=f"num_{m}")
                den = sp.tile([128, 1], f32, tag="den", bufs=8, name=f"den_{m}")
                nc.vector.tensor_tensor(out=num[:], in0=n0[:], in1=n1[:], op=OP.add)
                nc.vector.tensor_tensor(out=den[:], in0=r0[:], in1=r1[:], op=OP.add)
                inv = sp.tile([128, 1], f32, tag="inv", bufs=8, name=f"inv_{m}")
                nc.vector.reciprocal(inv[:], den[:])
                q = sp.tile([128, 1], f32, tag="q", bufs=8, name=f"q_{m}")
                nc.vector.tensor_tensor(out=q[:], in0=num[:], in1=inv[:], op=OP.mult)
                nc.vector.tensor_tensor(out=loss_sb[:, m:m + 1], in0=lse[:], in1=q[:],
                                        op=OP.subtract)

            ltp = ps.tile([128, 1024], f32, tag="ps", name="ltp")
            ident32 = pp.tile([128, 128], f32, tag="ident32")
            make_identity(nc, ident32[:])
            nc.tensor.transpose(ltp[:NM, 0:128], loss_sb[:], ident32[:])
            loss_t = pp.tile([4, 128], f32, tag="losst")
            nc.vector.tensor_copy(loss_t[:], ltp[:NM, 0:128])
            part = pp.tile([4, 1], f32, tag="part")
            nc.vector.reduce_sum(out=part[:], in_=loss_t[:], axis=mybir.AxisListType.X)
            nc.sync.dma_start(partial_d, part[:])
    nc.compile()
    return nc


_CACHE = {}


def _get_module():
    if "nc" not in _CACHE:
        nc = bacc.Bacc("TRN2", target_bir_lowering=False, debug=False,
                       num_devices=NCORES)
        _build(nc)
        _CACHE["nc"] = nc
    return _CACHE["nc"]


def _in_maps(inputs):
    import ml_dtypes
    hidden = np.asarray(inputs["hidden"], dtype=np.float32)
    Wc = np.asarray(inputs["Wc"], dtype=np.float32)
    bc = np.asarray(inputs["bc"], dtype=np.float32)
    Wp = np.asarray(inputs["Wp"], dtype=np.float32)
    bp = np.asarray(inputs["bp"], dtype=np.float32)
    Wo = np.asarray(inputs["Wo"], dtype=np.float32)
    bo = np.asarray(inputs["bo"], dtype=np.float32)
    discard = np.asarray(inputs["discard_probs"], dtype=np.float32)
    begins = np.asarray(inputs["begins"]).astype(np.int64)
    ends = np.asarray(inputs["ends"]).astype(np.int64)
    bids = np.asarray(inputs["bids"]).astype(np.int64)
    tags = np.asarray(inputs["tags"]).astype(np.int32)
    assert np.all(bo == 0.0), "kernel assumes bo == 0 (spec fill: zeros)"

    hid = np.ascontiguousarray(hidden.reshape(T * B, H2).astype(ml_dtypes.bfloat16))

    def pack_ffn(w):  # [D, H2] -> [128, 8*D] fp8,  w8[p, j*D+d] = w.T[j*128+p, d]
        wt = w.T.reshape(8, 128, D).transpose(1, 0, 2).reshape(128, 8 * D)
        return np.ascontiguousarray(wt.astype(ml_dtypes.float8_e4m3))

    wc8 = pack_ffn(Wc)
    wp8 = pack_ffn(0.5 * Wp)
    wot = np.ascontiguousarray((WOSCALE * Wo.T).astype(ml_dtypes.float8_e4m3))
    woaug = np.zeros((V, 514), dtype=ml_dtypes.bfloat16)
    woaug[:, 0:D] = Wo.astype(ml_dtypes.bfloat16)
    woaug[:, D] = discard.astype(ml_dtypes.bfloat16)
    woaug = np.ascontiguousarray(woaug)

    idx_lo = 2 * ((begins - 1) * B + bids)
    idx_hi = 2 * (ends * B + bids) + 1
    idx_beg = begins * B + bids
    idx_em1 = (ends - 1) * B + bids

    maps = []
    for c in range(NCORES):
        sl = slice(c * LSH, (c + 1) * LSH)
        gidx = np.stack([idx_lo[sl], idx_hi[sl], idx_beg[sl], idx_em1[sl]]
                        ).astype(np.int32)
        tsh = tags[sl]                                   # [LSH, K]
        tagidx = np.concatenate([tsh, tsh], axis=0).T.astype(np.int32)  # [K, ROWS]
        maps.append(dict(hid=hid, wc8=wc8, wp8=wp8, bc=bc, bp=bp, wot=wot,
                         woaug=woaug, gidx=np.ascontiguousarray(gidx),
                         tagidx=np.ascontiguousarray(tagidx)))
    return maps


def _run(inputs, trace=False):
    nc = _get_module()
    maps = _in_maps(inputs)
    res = bass_utils.run_bass_kernel_spmd(
        nc, maps, core_ids=list(range(NCORES)), trace=trace)
    total = sum(np.float64(res.results[c]["partial"].sum()) for c in range(NCORES))
    n = 2 * L
    out = np.array([total / (n + 1e-5)], dtype=np.float32)
    return out, res


def kernel(**inputs) -> np.ndarray:
    out, _ = _run(inputs, trace=False)
    return out
